# revision 31
# baseline (speedup 1.0000x reference)
"""nn_MultiHeadAttention kernel for 8 Trainium2 NeuronCores.

Sharding: 8 cores = 4 batches (data parallel) x 2 head-groups of 8 heads
(tensor parallel). Each core computes its batch's QKV projection for its
head group (column-parallel), RoPE, causal attention, and a partial
out-projection (row-parallel). Host sums the two partials per batch and
adds the output bias.

All matmul operands are bf16 (f32 PSUM accumulation); rel-err budget is
2e-2 so bf16 rounding (~0.5% of value scale) is comfortably inside it.
The partial out-projections are returned in bf16 too (summed in f32 on
host), halving writeback traffic.

DMA rules learned on HW: one DMA rides one engine (~22GB/s), so big
transfers are split into ~128-512KB pieces that spread across engines
and pipeline with their consumers; all weight matrices are pre-swizzled
on the host into the exact SBUF layouts so every weight load is a
contiguous full-rate transfer.

Per-core dataflow:
  Phase 1a: xT resident in SBUF (16 tile-DMAs). q,k produced transposed
    [D,T] per 128-channel m-tile; bias fused into the PSUM eviction (DVE
    tensor_scalar into the resident tile), then RoPE in place per m-tile
    (one swap-halves DMA pair + 3 in-place DVE ops).
  Phase 1b: v produced untransposed [T, GC] (bias via a K=1 matmul),
    evicted on DVE into 4-tile staging rows, scattered to a head-major
    DRAM buffer (contiguous 1KB lines), so each head's [k,d] tile reads
    back as 4 contiguous chunk-DMAs that pipeline with the PV consumer.
  Phase 2 (per head): scores computed transposed S^T[k,q] from resident
    roped q,k; causal masking via identity-matmul of an additive mask;
    diagonal chunks narrowed to the causal trapezoid (matmuls, mask,
    exp, PV, Z all skip dead columns); softmax without max-subtraction;
    exp on ScalarE from PSUM (scale fused); Z via ones-column matmul;
    P@V into outT[d,q]; normalization via DVE reciprocal + GPSIMD
    broadcast + DVE multiply into resident attnT.
  Phase 3: out-projection; bf16 output staged 2 m-tiles per DMA.
"""

import sys

if "/opt/trn_rl_repo" not in sys.path:
    sys.path.insert(0, "/opt/trn_rl_repo")

import numpy as np
import ml_dtypes

import concourse.bass as bass
import concourse.bacc as bacc
import concourse.mybir as mybir
import concourse.tile as tile
from concourse.bass_utils import run_bass_kernel_spmd

F32 = mybir.dt.float32
BF16 = mybir.dt.bfloat16
NPBF16 = ml_dtypes.bfloat16

B, T, C = 4, 2048, 2048
H = 16            # total heads
HG = 8            # heads per core (group)
D = 128           # head dim
GC = HG * D       # channels per group = 1024
SCALE = 1.0 / float(np.sqrt(D))
MASKVAL = -30000.0
N_CORES = 8

KT = C // 128     # 16 K tiles
TT = T // 128     # 16 T tiles
TC = T // 512     # 4 T chunks of 512

# diagonal-narrowing layout: r = position of the k-tile within the
# diagonal 512-block; valid q columns are [128r, 512) (width 512-128r).
# Tiles r0,r1 share one psum tile (cols 0:512 and 512:896); r2,r3 share
# a second (cols 0:256 and 256:384) so each exp is one contiguous ACT.
DIAG_W = [512, 384, 256, 128]
DIAG_TILE = [0, 0, 1, 1]          # which diag psum tile holds r
DIAG_OFF = [0, 512, 0, 256]       # column offset inside that tile


def build_program(iters=1):
    nc = bacc.Bacc("TRN2", target_bir_lowering=False, debug=False)

    # host-preswizzled tensors (see make_host_inputs):
    #  xTs   [128, KT*T]   col k*T + t          (x transposed tiles)
    #  wqk_t [128, 16*C]   col m*C + k*128 + c  (per-m wrow layout)
    #  wv_t  [128, 4*4096] col nd*4096 + k*256 + c
    #  wo_t  [128, 4*4096] col n*4096 + h*512 + c
    xTs = nc.dram_tensor("xTs", [128, KT * T], BF16, kind="ExternalInput").ap()
    wqk_t = nc.dram_tensor("wqk_t", [128, 16 * C], BF16, kind="ExternalInput").ap()
    wv_t = nc.dram_tensor("wv_t", [128, 4 * 4096], BF16, kind="ExternalInput").ap()
    wo_t = nc.dram_tensor("wo_t", [128, 4 * 4096], BF16, kind="ExternalInput").ap()
    bqk = nc.dram_tensor("bqk", [128, 16], F32, kind="ExternalInput").ap()
    bv = nc.dram_tensor("bv", [1, GC], BF16, kind="ExternalInput").ap()
    sin2 = nc.dram_tensor("sin2", [128, T], BF16, kind="ExternalInput").ap()
    cos2 = nc.dram_tensor("cos2", [128, T], BF16, kind="ExternalInput").ap()
    masks = nc.dram_tensor("masks", [4, 128, 512], BF16, kind="ExternalInput").ap()
    ident = nc.dram_tensor("ident", [128, 128], BF16, kind="ExternalInput").ap()
    onescol = nc.dram_tensor("onescol", [128, 1], BF16, kind="ExternalInput").ap()
    ones128 = nc.dram_tensor("ones128", [1, 128], BF16, kind="ExternalInput").ap()
    y = nc.dram_tensor("y", [T, C], BF16, kind="ExternalOutput").ap()

    with tile.TileContext(nc) as tc:
        with tc.tile_pool(name="dram", bufs=1, space="DRAM") as dpool, \
             tc.tile_pool(name="consts", bufs=1) as rpool:
            # head-major DRAM v: col = h*T + t*128 + d
            v_one = dpool.tile([128, HG * T], BF16, tag="vone", name="v_one")
            onescol_sb = rpool.tile([128, 1], BF16, tag="onescol")
            ones128_sb = rpool.tile([1, 128], BF16, tag="ones128")
            bv_sb = rpool.tile([1, GC], BF16, tag="bv")
            bqk_sb = rpool.tile([128, 16], F32, tag="bqk")
            masks_sb = rpool.tile([128, 4 * 512], BF16, tag="masks")
            ident_sb = rpool.tile([128, 128], BF16, tag="ident")
            warm_sb = rpool.tile([1, 1], F32, tag="warm")

            def full_body(iv):
                nc.sync.dma_start(out=onescol_sb[:], in_=onescol)
                nc.sync.dma_start(
                    out=masks_sb[:].rearrange("p (r c) -> p r c", r=4),
                    in_=masks.rearrange("r p c -> p r c"),
                )
                nc.sync.dma_start(out=ident_sb[:], in_=ident)
                nc.sync.dma_start(out=ones128_sb[:], in_=ones128)
                nc.sync.dma_start(out=bv_sb[:], in_=bv)
                nc.sync.dma_start(out=bqk_sb[:], in_=bqk)
                # preload the exp table set while phase 1 runs
                nc.scalar.activation(
                    warm_sb[:], onescol_sb[0:1, 0:1],
                    mybir.ActivationFunctionType.Exp,
                )

                with tc.tile_pool(name="attn", bufs=1) as apool:
                    attn_sb = [
                        apool.tile([128, T], BF16, tag=f"attn{h}", name=f"attn{h}")
                        for h in range(HG)
                    ]

                    with tc.tile_pool(name="qkro", bufs=1) as qkpool:
                        qk_ro = [
                            qkpool.tile([128, T], BF16, tag=f"qk{m}", name=f"qk{m}")
                            for m in range(16)
                        ]

                        # ---------------- Phase 1: QKV projection ----------
                        with tc.tile_pool(name="p1x", bufs=1) as xpool, \
                             tc.tile_pool(name="p1ps", bufs=3, space="PSUM") as pspool:
                            xt = xpool.tile([128, KT * T], BF16, tag="xt", name="xt")
                            for k in range(KT):
                                nc.sync.dma_start(
                                    out=xt[:, k * T:(k + 1) * T],
                                    in_=xTs[:, k * T:(k + 1) * T],
                                )

                            def xsl(k, lo, hi):
                                return xt[:, k * T + lo: k * T + hi]

                            # q,k: 16 M-tiles of 128 channels, [D,T] layout,
                            # bias + RoPE fused per m-tile
                            with tc.tile_pool(name="p1w", bufs=2) as wpool, \
                                 tc.tile_pool(name="p1sc", bufs=1) as scpool, \
                                 tc.tile_pool(name="p1t", bufs=2) as tpool:
                                sin_sb = scpool.tile([128, T], BF16, tag="sin")
                                cos_sb = scpool.tile([128, T], BF16, tag="cos")
                                nc.sync.dma_start(out=sin_sb[:], in_=sin2)
                                nc.sync.dma_start(out=cos_sb[:], in_=cos2)
                                for m in range(16):
                                    wrow = wpool.tile([128, KT * 128], BF16,
                                                      tag="wrow")
                                    # 4-way split so the load spreads across
                                    # DMA engines (one DMA = one engine)
                                    for q4 in range(4):
                                        nc.sync.dma_start(
                                            out=wrow[:, q4 * 512:(q4 + 1) * 512],
                                            in_=wqk_t[:, m * C + q4 * 512:
                                                      m * C + (q4 + 1) * 512],
                                        )
                                    for n in range(TC):
                                        nsl = slice(n * 512, (n + 1) * 512)
                                        ps = pspool.tile([128, 512], F32,
                                                         tag="pqk")
                                        for k in range(KT):
                                            nc.tensor.matmul(
                                                ps[:],
                                                wrow[:, k * 128:(k + 1) * 128],
                                                xsl(k, n * 512, (n + 1) * 512),
                                                start=(k == 0),
                                                stop=(k == KT - 1),
                                            )
                                        raw = tpool.tile([128, 512], BF16,
                                                         tag="qraw")
                                        nc.vector.tensor_scalar_add(
                                            raw[:], ps[:], bqk_sb[:, m:m + 1]
                                        )
                                        # swap-halves copy for the rotate, then
                                        # ro = raw*cos + sw*[-sin;+sin]
                                        sw = tpool.tile([128, 512], BF16,
                                                        tag="qsw")
                                        nc.sync.dma_start(
                                            out=sw[0:64, :], in_=raw[64:128, :]
                                        )
                                        nc.sync.dma_start(
                                            out=sw[64:128, :], in_=raw[0:64, :]
                                        )
                                        nc.vector.tensor_mul(
                                            sw[:], sw[:], sin_sb[:, nsl]
                                        )
                                        nc.vector.tensor_mul(
                                            raw[:], raw[:], cos_sb[:, nsl]
                                        )
                                        nc.vector.tensor_add(
                                            qk_ro[m][:, nsl], raw[:], sw[:]
                                        )

                            # v: [T, GC] untransposed, N=256 chunks, bias via
                            # K=1 matmul; evicted via 4-tile staging into
                            # head-major v_one
                            with tc.tile_pool(name="p1wv", bufs=2) as wvpool, \
                                 tc.tile_pool(name="p1v", bufs=2) as vspool:
                                for nd in range(4):
                                    ndsl = slice(nd * 256, (nd + 1) * 256)
                                    wvc = wvpool.tile([128, KT * 256], BF16,
                                                      tag="wvc")
                                    for q4 in range(4):
                                        nc.sync.dma_start(
                                            out=wvc[:, q4 * 1024:
                                                    (q4 + 1) * 1024],
                                            in_=wv_t[:, nd * 4096 + q4 * 1024:
                                                     nd * 4096 +
                                                     (q4 + 1) * 1024],
                                        )
                                    for tb in range(4):
                                        vrow = vspool.tile([128, 4 * 256], BF16,
                                                           tag="vrow")
                                        for i in range(4):
                                            t = 4 * tb + i
                                            ps = pspool.tile([128, 256], F32,
                                                             tag="pv")
                                            for k in range(KT):
                                                nc.tensor.matmul(
                                                    ps[:],
                                                    xsl(k, t * 128,
                                                        (t + 1) * 128),
                                                    wvc[:, k * 256:
                                                        (k + 1) * 256],
                                                    start=(k == 0),
                                                    stop=False,
                                                )
                                            nc.tensor.matmul(
                                                ps[:],
                                                ones128_sb[:],
                                                bv_sb[:, ndsl],
                                                start=False,
                                                stop=True,
                                            )
                                            nc.vector.tensor_copy(
                                                vrow[:, i * 256:(i + 1) * 256],
                                                ps[:],
                                            )
                                        # scatter: cols (2nd+hh)*T + (4tb+i)*128
                                        # + d, per head hh of the 2 in chunk
                                        for hh in range(2):
                                            hcol = ((2 * nd + hh) * T
                                                    + 4 * tb * 128)
                                            nc.sync.dma_start(
                                                out=v_one[:, hcol:hcol + 512]
                                                .rearrange(
                                                    "p (t d) -> p t d", t=4
                                                ),
                                                in_=vrow[:].rearrange(
                                                    "p (i hh d) -> p i hh d",
                                                    i=4, hh=2
                                                )[:, :, hh, :],
                                            )

                        # ---- Phase 2: attention per head ----
                        with tc.tile_pool(name="p2v", bufs=2) as vpool, \
                             tc.tile_pool(name="p2e", bufs=4) as epool, \
                             tc.tile_pool(name="p2n", bufs=2) as npool, \
                             tc.tile_pool(name="p2ps", bufs=2, space="PSUM") as ps2, \
                             tc.tile_pool(name="p2po", bufs=2, space="PSUM") as po2:
                            for h in range(HG):
                                qr = qk_ro[h]
                                kr = qk_ro[8 + h]
                                vh = vpool.tile([128, T], BF16, tag="vh")
                                for c in range(4):
                                    nc.sync.dma_start(
                                        out=vh[:, c * 512:(c + 1) * 512],
                                        in_=v_one[:, h * T + c * 512:
                                                  h * T + (c + 1) * 512],
                                    )
                                for n in range(TC):
                                    ps_o = po2.tile([128, 512], F32, tag="po")
                                    ps_z = po2.tile([1, 512], F32, tag="pz")
                                    qsl = slice(n * 512, (n + 1) * 512)
                                    # full (non-diagonal) k-tiles in pairs:
                                    # two score chunks into one 2-bank psum
                                    # tile, ONE 1024-wide exp
                                    pexps = []   # (pexp_ap, col_off, width, qo, j)
                                    for jp in range(2 * n):
                                        ps_s = ps2.tile([128, 1024], F32, tag="ps")
                                        for u in range(2):
                                            j = 2 * jp + u
                                            half = slice(u * 512, (u + 1) * 512)
                                            nc.tensor.matmul(
                                                ps_s[:, half],
                                                kr[:, j * 128:(j + 1) * 128],
                                                qr[:, qsl],
                                                start=True,
                                                stop=True,
                                            )
                                        pexp = epool.tile(
                                            [128, 1024], BF16, tag="pexp"
                                        )
                                        nc.scalar.activation(
                                            pexp[:],
                                            ps_s[:],
                                            mybir.ActivationFunctionType.Exp,
                                            scale=SCALE,
                                        )
                                        for u in range(2):
                                            j = 2 * jp + u
                                            pexps.append(
                                                (pexp, u * 512, 512, 0, j)
                                            )
                                    # diagonal group: narrowed to the causal
                                    # trapezoid
                                    ps_d = [
                                        ps2.tile([128, 1024], F32, tag="ps",
                                                 name="psd0"),
                                        ps2.tile([128, 1024], F32, tag="ps",
                                                 name="psd1"),
                                    ]
                                    for r in range(4):
                                        j = 4 * n + r
                                        w_r = DIAG_W[r]
                                        dsl = slice(DIAG_OFF[r], DIAG_OFF[r] + w_r)
                                        nc.tensor.matmul(
                                            ps_d[DIAG_TILE[r]][:, dsl],
                                            kr[:, j * 128:(j + 1) * 128],
                                            qr[:, n * 512 + 128 * r:(n + 1) * 512],
                                            start=True,
                                            stop=False,
                                        )
                                        nc.tensor.matmul(
                                            ps_d[DIAG_TILE[r]][:, dsl],
                                            ident_sb[:],
                                            masks_sb[:, r * 512 + 128 * r:
                                                     (r + 1) * 512],
                                            start=False,
                                            stop=True,
                                        )
                                    pexp_d = [
                                        epool.tile([128, 1024], BF16, tag="pexp",
                                                   name="pexpd0"),
                                        epool.tile([128, 1024], BF16, tag="pexp",
                                                   name="pexpd1"),
                                    ]
                                    nc.scalar.activation(
                                        pexp_d[0][:, 0:896],
                                        ps_d[0][:, 0:896],
                                        mybir.ActivationFunctionType.Exp,
                                        scale=SCALE,
                                    )
                                    nc.scalar.activation(
                                        pexp_d[1][:, 0:384],
                                        ps_d[1][:, 0:384],
                                        mybir.ActivationFunctionType.Exp,
                                        scale=SCALE,
                                    )
                                    for r in range(4):
                                        pexps.append(
                                            (pexp_d[DIAG_TILE[r]], DIAG_OFF[r],
                                             DIAG_W[r], 128 * r, 4 * n + r)
                                        )
                                    # P@V and Z accumulation
                                    njj = len(pexps)
                                    for idx, (pe_t, off, w_r, qo, j) in enumerate(
                                        pexps
                                    ):
                                        psl = slice(off, off + w_r)
                                        osl = slice(qo, 512)
                                        nc.tensor.matmul(
                                            ps_o[:, osl],
                                            vh[:, j * 128:(j + 1) * 128],
                                            pe_t[:, psl],
                                            start=(idx == 0),
                                            stop=(idx == njj - 1),
                                        )
                                        nc.tensor.matmul(
                                            ps_z[:, osl],
                                            onescol_sb[:],
                                            pe_t[:, psl],
                                            start=(idx == 0),
                                            stop=(idx == njj - 1),
                                        )
                                    rz = npool.tile([1, 512], F32, tag="rz")
                                    nc.vector.reciprocal(rz[:], ps_z[:])
                                    rzb = npool.tile([128, 512], F32, tag="rzb")
                                    nc.gpsimd.partition_broadcast(rzb[:], rz[:])
                                    nc.vector.tensor_mul(
                                        attn_sb[h][:, qsl],
                                        ps_o[:],
                                        rzb[:],
                                    )

                    # ---- Phase 3: out projection ----
                    with tc.tile_pool(name="p3w", bufs=2) as wpool3, \
                         tc.tile_pool(name="p3t", bufs=3) as tpool3, \
                         tc.tile_pool(name="p3ps", bufs=2, space="PSUM") as ps3:
                        for n in range(4):
                            woc = wpool3.tile([128, HG * 512], BF16, tag="woc")
                            for q4 in range(4):
                                nc.sync.dma_start(
                                    out=woc[:, q4 * 1024:(q4 + 1) * 1024],
                                    in_=wo_t[:, n * 4096 + q4 * 1024:
                                             n * 4096 + (q4 + 1) * 1024],
                                )
                            for mb in range(8):
                                yrow = tpool3.tile([128, 2 * 512], BF16, tag="yt")
                                for i in range(2):
                                    m = 2 * mb + i
                                    ps_y = ps3.tile([128, 512], F32, tag="py")
                                    for h in range(HG):
                                        nc.tensor.matmul(
                                            ps_y[:],
                                            attn_sb[h][:, m * 128:(m + 1) * 128],
                                            woc[:, h * 512:(h + 1) * 512],
                                            start=(h == 0),
                                            stop=(h == HG - 1),
                                        )
                                    nc.scalar.copy(
                                        yrow[:, i * 512:(i + 1) * 512], ps_y[:]
                                    )
                                nc.sync.dma_start(
                                    out=y[mb * 256:(mb + 1) * 256,
                                          n * 512:(n + 1) * 512].rearrange(
                                        "(i p) c -> p i c", p=128
                                    ),
                                    in_=yrow[:].rearrange(
                                        "p (i c) -> p i c", i=2
                                    ),
                                )

            if iters <= 4:
                for _ in range(iters):
                    full_body(None)
            else:
                with tc.For_i(0, iters, 1) as iv:
                    full_body(iv)

    nc.compile()
    return nc


def make_host_inputs(x, Wqkv, bqkv, Wo):
    """Per-core input maps (host-side sharding + weight swizzling)."""
    half = D // 2
    freq = np.arange(half, dtype=np.float64)
    theta = 1.0 / (10000.0 ** (2.0 * freq / D))
    pos = np.arange(T, dtype=np.float64)
    ang = pos[:, None] * theta[None, :]          # [T, half]
    sinT = np.sin(ang).T.astype(np.float32)      # [half, T]
    cosT = np.cos(ang).T.astype(np.float32)
    # sign folded into the sin table for the partition-swap RoPE form
    sin2 = np.concatenate([-sinT, sinT], axis=0).astype(NPBF16)  # [128, T]
    cos2 = np.concatenate([cosT, cosT], axis=0).astype(NPBF16)

    masks = np.zeros((4, 128, 512), dtype=np.float32)
    f = np.arange(512)[None, :]
    p = np.arange(128)[:, None]
    for r in range(4):
        masks[r] = np.where(f >= r * 128 + p, 0.0, MASKVAL)
    masks = masks.astype(NPBF16)
    ident = np.eye(128, dtype=np.float32).astype(NPBF16)
    onescol = np.ones((128, 1), dtype=NPBF16)
    ones128 = np.ones((1, 128), dtype=NPBF16)

    xT = [np.ascontiguousarray(x[b].T).astype(NPBF16) for b in range(B)]
    in_maps = []
    for core in range(N_CORES):
        b, g = core // 2, core % 2
        cs = slice(g * GC, (g + 1) * GC)
        wq = Wqkv[:, :C][:, cs]          # [C, GC]
        wk = Wqkv[:, C:2 * C][:, cs]
        wv = Wqkv[:, 2 * C:][:, cs]
        wo = Wo[cs, :]                   # [GC, C]

        # xTs [128, KT*T]: col k*T + t, row p; element x^T[k*128+p, t]
        xts = xT[b].reshape(KT, 128, T).transpose(1, 0, 2).reshape(128, KT * T)

        # wqk_t [128, 16*C]: m-tile m (q: m<8 chans m*128.., k: m>=8),
        # col m*C + k*128 + c = w[k*128+p, row*128+c]
        wqk = np.empty((128, 16 * C), dtype=np.float32)
        for m in range(16):
            w = wq if m < 8 else wk
            row = m % 8
            blk = w[:, row * 128:(row + 1) * 128]       # [C, 128]
            wqk[:, m * C:(m + 1) * C] = (
                blk.reshape(KT, 128, 128).transpose(1, 0, 2).reshape(128, C)
            )

        # wv_t [128, 4*4096]: col nd*4096 + k*256 + c = wv[k*128+p, nd*256+c]
        wvt = np.empty((128, 4 * 4096), dtype=np.float32)
        for nd in range(4):
            blk = wv[:, nd * 256:(nd + 1) * 256]        # [C, 256]
            wvt[:, nd * 4096:(nd + 1) * 4096] = (
                blk.reshape(KT, 128, 256).transpose(1, 0, 2).reshape(128, 4096)
            )

        # wo_t [128, 4*4096]: col n*4096 + h*512 + c = wo[h*128+p, n*512+c]
        wot = np.empty((128, 4 * 4096), dtype=np.float32)
        for n in range(4):
            blk = wo[:, n * 512:(n + 1) * 512]          # [GC, 512]
            wot[:, n * 4096:(n + 1) * 4096] = (
                blk.reshape(HG, 128, 512).transpose(1, 0, 2).reshape(128, 4096)
            )

        # bqk [128, 16]: col m -> bias[row*128+p]
        bqs = bqkv[:C][cs].reshape(8, 128).T            # [128, 8]
        bks = bqkv[C:2 * C][cs].reshape(8, 128).T
        bqk = np.concatenate([bqs, bks], axis=1)        # [128, 16]

        in_maps.append({
            "xTs": np.ascontiguousarray(xts),
            "wqk_t": np.ascontiguousarray(wqk).astype(NPBF16),
            "wv_t": np.ascontiguousarray(wvt).astype(NPBF16),
            "wo_t": np.ascontiguousarray(wot).astype(NPBF16),
            "bqk": np.ascontiguousarray(bqk).astype(np.float32),
            "bv": np.ascontiguousarray(
                bqkv[2 * C:][cs].reshape(1, GC)).astype(NPBF16),
            "sin2": sin2,
            "cos2": cos2,
            "masks": masks,
            "ident": ident,
            "onescol": onescol,
            "ones128": ones128,
        })
    return in_maps


_PROGRAM_CACHE = {}


def get_program(iters=1):
    if iters not in _PROGRAM_CACHE:
        _PROGRAM_CACHE[iters] = build_program(iters)
    return _PROGRAM_CACHE[iters]


def kernel(x, Wqkv, bqkv, Wo, bo):
    x = np.asarray(x, dtype=np.float32)
    Wqkv = np.asarray(Wqkv, dtype=np.float32)
    bqkv = np.asarray(bqkv, dtype=np.float32)
    Wo = np.asarray(Wo, dtype=np.float32)
    bo = np.asarray(bo, dtype=np.float32)

    nc = get_program(1)
    in_maps = make_host_inputs(x, Wqkv, bqkv, Wo)
    res = run_bass_kernel_spmd(nc, in_maps, list(range(N_CORES)))

    out = np.empty((B, T, C), dtype=np.float32)
    for b in range(B):
        out[b] = (res.results[2 * b]["y"].astype(np.float32)
                  + res.results[2 * b + 1]["y"].astype(np.float32) + bo)
    return out


# revision 32
# speedup vs baseline: 1.3746x; 1.3746x over previous
"""nn_MultiHeadAttention kernel for 8 Trainium2 NeuronCores.

Sharding: 8 cores = 4 batches (data parallel) x 2 head-groups of 8 heads
(tensor parallel). Each core computes its batch's QKV projection for its
head group (column-parallel), RoPE, causal attention, and a partial
out-projection (row-parallel). Host sums the two partials per batch and
adds the output bias.

All matmul operands are bf16 (f32 PSUM accumulation); rel-err budget is
2e-2 so bf16 rounding (~0.5% of value scale) is comfortably inside it.

Per-core dataflow:
  Phase 1a: xT resident in SBUF. q,k produced transposed [D,T]; bias is
    fused into the PSUM eviction (DVE tensor_scalar) and RoPE is applied
    immediately per 512-chunk (swap-halves via two SBUF->SBUF DMAs, then
    3 DVE ops) so roped q,k land in SBUF-resident tiles - no DRAM round
    trip and the RoPE DVE work hides under phase-1 matmuls.
  Phase 1b: v produced untransposed [T, GC] (bias via a K=1 matmul),
    evicted on DVE, spilled to DRAM (SBUF is too tight for v residency).
  Phase 2 (per head): v gathered back [k,d] per head (prefetched);
    scores computed transposed S^T[k,q] directly from the resident roped
    q,k; causal masking of diagonal chunks via identity-matmul of a
    precomputed additive mask; diagonal chunks are narrowed to the valid
    causal trapezoid (matmuls, mask, exp, PV and Z all skip the dead
    columns); softmax without max-subtraction (scores ~N(0,1)); exp on
    ScalarE from PSUM (scale fused); Z via a ones-column matmul; P@V
    accumulated into outT[d,q]; normalization by DVE reciprocal + GPSIMD
    partition-broadcast + DVE multiply on eviction into resident attnT.
  Phase 3: out-projection from the SBUF-resident attnT tiles.
"""

import sys

if "/opt/trn_rl_repo" not in sys.path:
    sys.path.insert(0, "/opt/trn_rl_repo")

import numpy as np
import ml_dtypes

import concourse.bass as bass
import concourse.bacc as bacc
import concourse.mybir as mybir
import concourse.tile as tile
from concourse.bass_utils import run_bass_kernel_spmd

F32 = mybir.dt.float32
BF16 = mybir.dt.bfloat16
NPBF16 = ml_dtypes.bfloat16

B, T, C = 4, 2048, 2048
H = 16            # total heads
HG = 8            # heads per core (group)
D = 128           # head dim
GC = HG * D       # channels per group = 1024
SCALE = 1.0 / float(np.sqrt(D))
MASKVAL = -30000.0
N_CORES = 8

KT = C // 128     # 16 K tiles
TT = T // 128     # 16 T tiles
TC = T // 512     # 4 T chunks of 512

# diagonal-narrowing layout: r = position of the k-tile within the
# diagonal 512-block; valid q columns are [128r, 512) (width 512-128r).
# Tiles r0,r1 share one psum tile (cols 0:512 and 512:896); r2,r3 share
# a second (cols 0:256 and 256:384) so each exp is one contiguous ACT.
DIAG_W = [512, 384, 256, 128]
DIAG_TILE = [0, 0, 1, 1]          # which diag psum tile holds r
DIAG_OFF = [0, 512, 0, 256]       # column offset inside that tile


def build_program(iters=1):
    nc = bacc.Bacc("TRN2", target_bir_lowering=False, debug=False)

    xT = nc.dram_tensor("xT", [C, T], BF16, kind="ExternalInput").ap()
    wq = nc.dram_tensor("wq", [C, GC], BF16, kind="ExternalInput").ap()
    wk = nc.dram_tensor("wk", [C, GC], BF16, kind="ExternalInput").ap()
    wv = nc.dram_tensor("wv", [C, GC], BF16, kind="ExternalInput").ap()
    bq = nc.dram_tensor("bq", [GC, 1], F32, kind="ExternalInput").ap()
    bk = nc.dram_tensor("bk", [GC, 1], F32, kind="ExternalInput").ap()
    bv = nc.dram_tensor("bv", [1, GC], BF16, kind="ExternalInput").ap()
    wo = nc.dram_tensor("wo", [GC, C], BF16, kind="ExternalInput").ap()
    sin2 = nc.dram_tensor("sin2", [128, T], BF16, kind="ExternalInput").ap()
    cos2 = nc.dram_tensor("cos2", [128, T], BF16, kind="ExternalInput").ap()
    masks = nc.dram_tensor("masks", [4, 128, 512], BF16, kind="ExternalInput").ap()
    ident = nc.dram_tensor("ident", [128, 128], BF16, kind="ExternalInput").ap()
    onescol = nc.dram_tensor("onescol", [128, 1], BF16, kind="ExternalInput").ap()
    ones128 = nc.dram_tensor("ones128", [1, 128], BF16, kind="ExternalInput").ap()
    y = nc.dram_tensor("y", [T, C], F32, kind="ExternalOutput").ap()

    with tile.TileContext(nc) as tc:
        with tc.tile_pool(name="dram", bufs=1, space="DRAM") as dpool, \
             tc.tile_pool(name="consts", bufs=1) as rpool:
            v_d = [dpool.tile([128, GC], BF16, tag=f"vd{t}", name=f"vd{t}")
                   for t in range(TT)]
            masks_sb = rpool.tile([128, 4 * 512], BF16, tag="masks")
            ident_sb = rpool.tile([128, 128], BF16, tag="ident")
            onescol_sb = rpool.tile([128, 1], BF16, tag="onescol")
            ones128_sb = rpool.tile([1, 128], BF16, tag="ones128")
            bv_sb = rpool.tile([1, GC], BF16, tag="bv")
            sin_sb = rpool.tile([128, T], BF16, tag="sin")
            cos_sb = rpool.tile([128, T], BF16, tag="cos")

            def full_body(iv):
                nc.sync.dma_start(
                    out=masks_sb[:].rearrange("p (r c) -> p r c", r=4),
                    in_=masks.rearrange("r p c -> p r c"),
                )
                nc.sync.dma_start(out=ident_sb[:], in_=ident)
                nc.sync.dma_start(out=onescol_sb[:], in_=onescol)
                nc.sync.dma_start(out=ones128_sb[:], in_=ones128)
                nc.sync.dma_start(out=bv_sb[:], in_=bv)
                nc.sync.dma_start(out=sin_sb[:], in_=sin2)
                nc.sync.dma_start(out=cos_sb[:], in_=cos2)

                with tc.tile_pool(name="attn", bufs=1) as apool:
                    attn_sb = [
                        apool.tile([128, T], BF16, tag=f"attn{h}", name=f"attn{h}")
                        for h in range(HG)
                    ]

                    with tc.tile_pool(name="qkro", bufs=1) as qkpool:
                        qk_ro = [
                            qkpool.tile([128, T], BF16, tag=f"qk{m}", name=f"qk{m}")
                            for m in range(16)
                        ]

                        # ---------------- Phase 1: QKV projection ----------
                        with tc.tile_pool(name="p1x", bufs=1) as xpool, \
                             tc.tile_pool(name="p1w", bufs=2) as wpool, \
                             tc.tile_pool(name="p1wv", bufs=2) as wvpool, \
                             tc.tile_pool(name="p1t", bufs=2) as tpool, \
                             tc.tile_pool(name="p1ps", bufs=2, space="PSUM") as pspool:
                            xt_sb = []
                            for k in range(KT):
                                t = xpool.tile([128, T], BF16,
                                               tag=f"xt{k}", name=f"xt{k}")
                                nc.sync.dma_start(
                                    out=t[:], in_=xT[k * 128:(k + 1) * 128, :]
                                )
                                xt_sb.append(t)

                            # q,k: 16 M-tiles of 128 channels, [D,T] layout,
                            # bias + RoPE fused at chunk granularity
                            for m in range(16):
                                w = wq if m < 8 else wk
                                bias = bq if m < 8 else bk
                                row = m % 8
                                wrow = wpool.tile([128, KT * 128], BF16, tag="wrow")
                                nc.sync.dma_start(
                                    out=wrow[:].rearrange("p (k c) -> p k c", k=KT),
                                    in_=w[:, row * 128:(row + 1) * 128].rearrange(
                                        "(k p) c -> p k c", p=128
                                    ),
                                )
                                bias_t = wpool.tile([128, 1], F32, tag="bias")
                                nc.sync.dma_start(
                                    out=bias_t[:],
                                    in_=bias[row * 128:(row + 1) * 128, :],
                                )
                                for n in range(TC):
                                    nsl = slice(n * 512, (n + 1) * 512)
                                    ps = pspool.tile([128, 512], F32, tag="pqk")
                                    for k in range(KT):
                                        nc.tensor.matmul(
                                            ps[:],
                                            wrow[:, k * 128:(k + 1) * 128],
                                            xt_sb[k][:, nsl],
                                            start=(k == 0),
                                            stop=(k == KT - 1),
                                        )
                                    raw = tpool.tile([128, 512], BF16, tag="qt")
                                    nc.vector.tensor_scalar_add(
                                        raw[:], ps[:], bias_t[:]
                                    )
                                    # swap-halves copy for the rotate
                                    sw = tpool.tile([128, 512], BF16, tag="qsw")
                                    nc.sync.dma_start(
                                        out=sw[0:64, :], in_=raw[64:128, :]
                                    )
                                    nc.sync.dma_start(
                                        out=sw[64:128, :], in_=raw[0:64, :]
                                    )
                                    # ro = raw*cos + sw*[-sin;+sin]
                                    ct = tpool.tile([128, 512], BF16, tag="qct")
                                    nc.vector.tensor_mul(
                                        sw[:], sw[:], sin_sb[:, nsl]
                                    )
                                    nc.vector.tensor_mul(
                                        ct[:], raw[:], cos_sb[:, nsl]
                                    )
                                    nc.vector.tensor_add(
                                        qk_ro[m][:, nsl], ct[:], sw[:]
                                    )

                            # v: [T, GC] untransposed, bias via K=1 matmul
                            for nd in range(4):
                                ndsl = slice(nd * 256, (nd + 1) * 256)
                                wvc = wvpool.tile([128, KT * 256], BF16, tag="wvc")
                                nc.sync.dma_start(
                                    out=wvc[:].rearrange("p (k c) -> p k c", k=KT),
                                    in_=wv[:, ndsl].rearrange(
                                        "(k p) c -> p k c", p=128
                                    ),
                                )
                                for t in range(TT):
                                    ps = pspool.tile([128, 256], F32, tag="pv")
                                    for k in range(KT):
                                        nc.tensor.matmul(
                                            ps[:],
                                            xt_sb[k][:, t * 128:(t + 1) * 128],
                                            wvc[:, k * 256:(k + 1) * 256],
                                            start=(k == 0),
                                            stop=False,
                                        )
                                    nc.tensor.matmul(
                                        ps[:],
                                        ones128_sb[:],
                                        bv_sb[:, ndsl],
                                        start=False,
                                        stop=True,
                                    )
                                    vt = tpool.tile([128, 256], BF16, tag="vt")
                                    nc.vector.tensor_copy(vt[:], ps[:])
                                    nc.sync.dma_start(out=v_d[t][:, ndsl], in_=vt[:])

                        # ---- Phase 2: attention per head ----
                        with tc.tile_pool(name="p2v", bufs=2) as vpool, \
                             tc.tile_pool(name="p2e", bufs=4) as epool, \
                             tc.tile_pool(name="p2n", bufs=2) as npool, \
                             tc.tile_pool(name="p2ps", bufs=2, space="PSUM") as ps2, \
                             tc.tile_pool(name="p2po", bufs=2, space="PSUM") as po2:
                            for h in range(HG):
                                qr = qk_ro[h]
                                kr = qk_ro[8 + h]
                                vh = vpool.tile([128, TT * 128], BF16, tag="vh")
                                for t in range(TT):
                                    nc.sync.dma_start(
                                        out=vh[:, t * 128:(t + 1) * 128],
                                        in_=v_d[t][:, h * 128:(h + 1) * 128],
                                    )
                                for n in range(TC):
                                    ps_o = po2.tile([128, 512], F32, tag="po")
                                    ps_z = po2.tile([1, 512], F32, tag="pz")
                                    qsl = slice(n * 512, (n + 1) * 512)
                                    # full (non-diagonal) k-tiles in pairs:
                                    # two score chunks into one 2-bank psum
                                    # tile, ONE 1024-wide exp
                                    pexps = []   # (pexp_ap, col_off, width, j)
                                    for jp in range(2 * n):
                                        ps_s = ps2.tile([128, 1024], F32, tag="ps")
                                        for u in range(2):
                                            j = 2 * jp + u
                                            half = slice(u * 512, (u + 1) * 512)
                                            nc.tensor.matmul(
                                                ps_s[:, half],
                                                kr[:, j * 128:(j + 1) * 128],
                                                qr[:, qsl],
                                                start=True,
                                                stop=True,
                                            )
                                        pexp = epool.tile(
                                            [128, 1024], BF16, tag="pexp"
                                        )
                                        nc.scalar.activation(
                                            pexp[:],
                                            ps_s[:],
                                            mybir.ActivationFunctionType.Exp,
                                            scale=SCALE,
                                        )
                                        for u in range(2):
                                            j = 2 * jp + u
                                            pexps.append(
                                                (pexp, u * 512, 512, 0, j)
                                            )
                                    # diagonal group: narrowed to the causal
                                    # trapezoid
                                    ps_d = [
                                        ps2.tile([128, 1024], F32, tag="ps",
                                                 name="psd0"),
                                        ps2.tile([128, 1024], F32, tag="ps",
                                                 name="psd1"),
                                    ]
                                    for r in range(4):
                                        j = 4 * n + r
                                        w_r = DIAG_W[r]
                                        dsl = slice(DIAG_OFF[r], DIAG_OFF[r] + w_r)
                                        nc.tensor.matmul(
                                            ps_d[DIAG_TILE[r]][:, dsl],
                                            kr[:, j * 128:(j + 1) * 128],
                                            qr[:, n * 512 + 128 * r:(n + 1) * 512],
                                            start=True,
                                            stop=False,
                                        )
                                        nc.tensor.matmul(
                                            ps_d[DIAG_TILE[r]][:, dsl],
                                            ident_sb[:],
                                            masks_sb[:, r * 512 + 128 * r:
                                                     (r + 1) * 512],
                                            start=False,
                                            stop=True,
                                        )
                                    pexp_d = [
                                        epool.tile([128, 1024], BF16, tag="pexp",
                                                   name="pexpd0"),
                                        epool.tile([128, 1024], BF16, tag="pexp",
                                                   name="pexpd1"),
                                    ]
                                    nc.scalar.activation(
                                        pexp_d[0][:, 0:896],
                                        ps_d[0][:, 0:896],
                                        mybir.ActivationFunctionType.Exp,
                                        scale=SCALE,
                                    )
                                    nc.scalar.activation(
                                        pexp_d[1][:, 0:384],
                                        ps_d[1][:, 0:384],
                                        mybir.ActivationFunctionType.Exp,
                                        scale=SCALE,
                                    )
                                    for r in range(4):
                                        pexps.append(
                                            (pexp_d[DIAG_TILE[r]], DIAG_OFF[r],
                                             DIAG_W[r], 128 * r, 4 * n + r)
                                        )
                                    # P@V and Z accumulation
                                    njj = len(pexps)
                                    for idx, (pe_t, off, w_r, qo, j) in enumerate(
                                        pexps
                                    ):
                                        psl = slice(off, off + w_r)
                                        osl = slice(qo, 512)
                                        nc.tensor.matmul(
                                            ps_o[:, osl],
                                            vh[:, j * 128:(j + 1) * 128],
                                            pe_t[:, psl],
                                            start=(idx == 0),
                                            stop=(idx == njj - 1),
                                        )
                                        nc.tensor.matmul(
                                            ps_z[:, osl],
                                            onescol_sb[:],
                                            pe_t[:, psl],
                                            start=(idx == 0),
                                            stop=(idx == njj - 1),
                                        )
                                    rz = npool.tile([1, 512], F32, tag="rz")
                                    nc.vector.reciprocal(rz[:], ps_z[:])
                                    rzb = npool.tile([128, 512], F32, tag="rzb")
                                    nc.gpsimd.partition_broadcast(rzb[:], rz[:])
                                    nc.vector.tensor_mul(
                                        attn_sb[h][:, qsl],
                                        ps_o[:],
                                        rzb[:],
                                    )

                    # ---- Phase 3: out projection ----
                    with tc.tile_pool(name="p3w", bufs=2) as wpool3, \
                         tc.tile_pool(name="p3t", bufs=3) as tpool3, \
                         tc.tile_pool(name="p3ps", bufs=2, space="PSUM") as ps3:
                        for n in range(4):
                            woc = wpool3.tile([128, HG * 512], BF16, tag="woc")
                            nc.sync.dma_start(
                                out=woc[:].rearrange("p (h c) -> p h c", h=HG),
                                in_=wo[:, n * 512:(n + 1) * 512].rearrange(
                                    "(h p) c -> p h c", p=128
                                ),
                            )
                            for m in range(TT):
                                ps_y = ps3.tile([128, 512], F32, tag="py")
                                for h in range(HG):
                                    nc.tensor.matmul(
                                        ps_y[:],
                                        attn_sb[h][:, m * 128:(m + 1) * 128],
                                        woc[:, h * 512:(h + 1) * 512],
                                        start=(h == 0),
                                        stop=(h == HG - 1),
                                    )
                                yt = tpool3.tile([128, 512], F32, tag="yt")
                                nc.scalar.copy(yt[:], ps_y[:])
                                nc.sync.dma_start(
                                    out=y[m * 128:(m + 1) * 128,
                                          n * 512:(n + 1) * 512],
                                    in_=yt[:],
                                )

            if iters == 1:
                full_body(None)
            else:
                with tc.For_i(0, iters, 1) as iv:
                    full_body(iv)

    nc.compile()
    return nc


def make_host_inputs(x, Wqkv, bqkv, Wo):
    """Per-core input maps (host-side sharding)."""
    half = D // 2
    freq = np.arange(half, dtype=np.float64)
    theta = 1.0 / (10000.0 ** (2.0 * freq / D))
    pos = np.arange(T, dtype=np.float64)
    ang = pos[:, None] * theta[None, :]          # [T, half]
    sinT = np.sin(ang).T.astype(np.float32)      # [half, T]
    cosT = np.cos(ang).T.astype(np.float32)
    # sign folded into the sin table for the partition-swap RoPE form
    sin2 = np.concatenate([-sinT, sinT], axis=0).astype(NPBF16)  # [128, T]
    cos2 = np.concatenate([cosT, cosT], axis=0).astype(NPBF16)

    masks = np.zeros((4, 128, 512), dtype=np.float32)
    f = np.arange(512)[None, :]
    p = np.arange(128)[:, None]
    for r in range(4):
        masks[r] = np.where(f >= r * 128 + p, 0.0, MASKVAL)
    masks = masks.astype(NPBF16)
    ident = np.eye(128, dtype=np.float32).astype(NPBF16)
    onescol = np.ones((128, 1), dtype=NPBF16)
    ones128 = np.ones((1, 128), dtype=NPBF16)

    xT = [np.ascontiguousarray(x[b].T).astype(NPBF16) for b in range(B)]
    in_maps = []
    for core in range(N_CORES):
        b, g = core // 2, core % 2
        cs = slice(g * GC, (g + 1) * GC)
        in_maps.append({
            "xT": xT[b],
            "wq": np.ascontiguousarray(Wqkv[:, :C][:, cs]).astype(NPBF16),
            "wk": np.ascontiguousarray(Wqkv[:, C:2 * C][:, cs]).astype(NPBF16),
            "wv": np.ascontiguousarray(Wqkv[:, 2 * C:][:, cs]).astype(NPBF16),
            "bq": np.ascontiguousarray(bqkv[:C][cs].reshape(GC, 1)),
            "bk": np.ascontiguousarray(bqkv[C:2 * C][cs].reshape(GC, 1)),
            "bv": np.ascontiguousarray(bqkv[2 * C:][cs].reshape(1, GC)).astype(NPBF16),
            "wo": np.ascontiguousarray(Wo[cs, :]).astype(NPBF16),
            "sin2": sin2,
            "cos2": cos2,
            "masks": masks,
            "ident": ident,
            "onescol": onescol,
            "ones128": ones128,
        })
    return in_maps


_PROGRAM_CACHE = {}


def get_program(iters=1):
    if iters not in _PROGRAM_CACHE:
        _PROGRAM_CACHE[iters] = build_program(iters)
    return _PROGRAM_CACHE[iters]


def kernel(x, Wqkv, bqkv, Wo, bo):
    x = np.asarray(x, dtype=np.float32)
    Wqkv = np.asarray(Wqkv, dtype=np.float32)
    bqkv = np.asarray(bqkv, dtype=np.float32)
    Wo = np.asarray(Wo, dtype=np.float32)
    bo = np.asarray(bo, dtype=np.float32)

    nc = get_program(1)
    in_maps = make_host_inputs(x, Wqkv, bqkv, Wo)
    res = run_bass_kernel_spmd(nc, in_maps, list(range(N_CORES)))

    out = np.empty((B, T, C), dtype=np.float32)
    for b in range(B):
        out[b] = res.results[2 * b]["y"] + res.results[2 * b + 1]["y"] + bo
    return out


# revision 33
# speedup vs baseline: 1.4448x; 1.0511x over previous
"""nn_MultiHeadAttention kernel for 8 Trainium2 NeuronCores.

Sharding: 8 cores = 4 batches (data parallel) x 2 head-groups of 8 heads
(tensor parallel). Each core computes its batch's QKV projection for its
head group (column-parallel), RoPE, causal attention, and a partial
out-projection (row-parallel). Host sums the two partials per batch and
adds the output bias.

All matmul operands are bf16 (f32 PSUM accumulation); rel-err budget is
2e-2 so bf16 rounding (~0.5% of value scale) is comfortably inside it.

Per-core dataflow:
  Phase 1a: xT resident in SBUF. q,k produced transposed [D,T]; bias is
    fused into the PSUM eviction (DVE tensor_scalar) and RoPE is applied
    immediately per 512-chunk (swap-halves via two SBUF->SBUF DMAs, then
    3 DVE ops) so roped q,k land in SBUF-resident tiles - no DRAM round
    trip and the RoPE DVE work hides under phase-1 matmuls.
  Phase 1b: v produced untransposed [T, GC] (bias via a K=1 matmul),
    evicted on DVE, spilled to DRAM (SBUF is too tight for v residency).
  Phase 2 (per head): v gathered back [k,d] per head (prefetched);
    scores computed transposed S^T[k,q] directly from the resident roped
    q,k; causal masking of diagonal chunks via identity-matmul of a
    precomputed additive mask; diagonal chunks are narrowed to the valid
    causal trapezoid (matmuls, mask, exp, PV and Z all skip the dead
    columns); softmax without max-subtraction (scores ~N(0,1)); exp on
    ScalarE from PSUM (scale fused); Z via a ones-column matmul; P@V
    accumulated into outT[d,q]; normalization by DVE reciprocal + GPSIMD
    partition-broadcast + DVE multiply on eviction into resident attnT.
  Phase 3: out-projection from the SBUF-resident attnT tiles.
"""

import sys

if "/opt/trn_rl_repo" not in sys.path:
    sys.path.insert(0, "/opt/trn_rl_repo")

import numpy as np
import ml_dtypes

import concourse.bass as bass
import concourse.bacc as bacc
import concourse.mybir as mybir
import concourse.tile as tile
from concourse.bass_utils import run_bass_kernel_spmd

F32 = mybir.dt.float32
BF16 = mybir.dt.bfloat16
NPBF16 = ml_dtypes.bfloat16

B, T, C = 4, 2048, 2048
H = 16            # total heads
HG = 8            # heads per core (group)
D = 128           # head dim
GC = HG * D       # channels per group = 1024
SCALE = 1.0 / float(np.sqrt(D))
MASKVAL = -30000.0
N_CORES = 8

KT = C // 128     # 16 K tiles
TT = T // 128     # 16 T tiles
TC = T // 512     # 4 T chunks of 512

# diagonal-narrowing layout: r = position of the k-tile within the
# diagonal 512-block; valid q columns are [128r, 512) (width 512-128r).
# Tiles r0,r1 share one psum tile (cols 0:512 and 512:896); r2,r3 share
# a second (cols 0:256 and 256:384) so each exp is one contiguous ACT.
DIAG_W = [512, 384, 256, 128]
DIAG_TILE = [0, 0, 1, 1]          # which diag psum tile holds r
DIAG_OFF = [0, 512, 0, 256]       # column offset inside that tile


def build_program(iters=1):
    nc = bacc.Bacc("TRN2", target_bir_lowering=False, debug=False)

    xT = nc.dram_tensor("xT", [C, T], BF16, kind="ExternalInput").ap()
    wq = nc.dram_tensor("wq", [C, GC], BF16, kind="ExternalInput").ap()
    wk = nc.dram_tensor("wk", [C, GC], BF16, kind="ExternalInput").ap()
    wv = nc.dram_tensor("wv", [C, GC], BF16, kind="ExternalInput").ap()
    bq = nc.dram_tensor("bq", [GC, 1], F32, kind="ExternalInput").ap()
    bk = nc.dram_tensor("bk", [GC, 1], F32, kind="ExternalInput").ap()
    bv = nc.dram_tensor("bv", [1, GC], BF16, kind="ExternalInput").ap()
    wo = nc.dram_tensor("wo", [GC, C], BF16, kind="ExternalInput").ap()
    sin2 = nc.dram_tensor("sin2", [128, T], BF16, kind="ExternalInput").ap()
    cos2 = nc.dram_tensor("cos2", [128, T], BF16, kind="ExternalInput").ap()
    masks = nc.dram_tensor("masks", [4, 128, 512], BF16, kind="ExternalInput").ap()
    ident = nc.dram_tensor("ident", [128, 128], BF16, kind="ExternalInput").ap()
    onescol = nc.dram_tensor("onescol", [128, 1], BF16, kind="ExternalInput").ap()
    ones128 = nc.dram_tensor("ones128", [1, 128], BF16, kind="ExternalInput").ap()
    y = nc.dram_tensor("y", [T, C], BF16, kind="ExternalOutput").ap()

    with tile.TileContext(nc) as tc:
        with tc.tile_pool(name="dram", bufs=1, space="DRAM") as dpool, \
             tc.tile_pool(name="consts", bufs=1) as rpool:
            v_d = [dpool.tile([128, GC], BF16, tag=f"vd{t}", name=f"vd{t}")
                   for t in range(TT)]
            masks_sb = rpool.tile([128, 4 * 512], BF16, tag="masks")
            ident_sb = rpool.tile([128, 128], BF16, tag="ident")
            onescol_sb = rpool.tile([128, 1], BF16, tag="onescol")
            ones128_sb = rpool.tile([1, 128], BF16, tag="ones128")
            bv_sb = rpool.tile([1, GC], BF16, tag="bv")
            sin_sb = rpool.tile([128, T], BF16, tag="sin")
            cos_sb = rpool.tile([128, T], BF16, tag="cos")

            def full_body(iv):
                nc.sync.dma_start(
                    out=masks_sb[:].rearrange("p (r c) -> p r c", r=4),
                    in_=masks.rearrange("r p c -> p r c"),
                )
                nc.sync.dma_start(out=ident_sb[:], in_=ident)
                nc.sync.dma_start(out=onescol_sb[:], in_=onescol)
                nc.sync.dma_start(out=ones128_sb[:], in_=ones128)
                nc.sync.dma_start(out=bv_sb[:], in_=bv)
                nc.sync.dma_start(out=sin_sb[:], in_=sin2)
                nc.sync.dma_start(out=cos_sb[:], in_=cos2)

                with tc.tile_pool(name="attn", bufs=1) as apool:
                    attn_sb = [
                        apool.tile([128, T], BF16, tag=f"attn{h}", name=f"attn{h}")
                        for h in range(HG)
                    ]

                    with tc.tile_pool(name="qkro", bufs=1) as qkpool:
                        qk_ro = [
                            qkpool.tile([128, T], BF16, tag=f"qk{m}", name=f"qk{m}")
                            for m in range(16)
                        ]

                        # ---------------- Phase 1: QKV projection ----------
                        with tc.tile_pool(name="p1x", bufs=1) as xpool, \
                             tc.tile_pool(name="p1w", bufs=2) as wpool, \
                             tc.tile_pool(name="p1wv", bufs=2) as wvpool, \
                             tc.tile_pool(name="p1t", bufs=2) as tpool, \
                             tc.tile_pool(name="p1ps", bufs=2, space="PSUM") as pspool:
                            xt_sb = []
                            for k in range(KT):
                                t = xpool.tile([128, T], BF16,
                                               tag=f"xt{k}", name=f"xt{k}")
                                nc.sync.dma_start(
                                    out=t[:], in_=xT[k * 128:(k + 1) * 128, :]
                                )
                                xt_sb.append(t)

                            # q,k: 16 M-tiles of 128 channels, [D,T] layout,
                            # bias + RoPE fused at chunk granularity
                            for m in range(16):
                                w = wq if m < 8 else wk
                                bias = bq if m < 8 else bk
                                row = m % 8
                                wrow = wpool.tile([128, KT * 128], BF16, tag="wrow")
                                nc.sync.dma_start(
                                    out=wrow[:].rearrange("p (k c) -> p k c", k=KT),
                                    in_=w[:, row * 128:(row + 1) * 128].rearrange(
                                        "(k p) c -> p k c", p=128
                                    ),
                                )
                                bias_t = wpool.tile([128, 1], F32, tag="bias")
                                nc.sync.dma_start(
                                    out=bias_t[:],
                                    in_=bias[row * 128:(row + 1) * 128, :],
                                )
                                for n in range(TC):
                                    nsl = slice(n * 512, (n + 1) * 512)
                                    ps = pspool.tile([128, 512], F32, tag="pqk")
                                    for k in range(KT):
                                        nc.tensor.matmul(
                                            ps[:],
                                            wrow[:, k * 128:(k + 1) * 128],
                                            xt_sb[k][:, nsl],
                                            start=(k == 0),
                                            stop=(k == KT - 1),
                                        )
                                    raw = tpool.tile([128, 512], BF16, tag="qt")
                                    nc.vector.tensor_scalar_add(
                                        raw[:], ps[:], bias_t[:]
                                    )
                                    # swap-halves copy for the rotate
                                    sw = tpool.tile([128, 512], BF16, tag="qsw")
                                    nc.sync.dma_start(
                                        out=sw[0:64, :], in_=raw[64:128, :]
                                    )
                                    nc.sync.dma_start(
                                        out=sw[64:128, :], in_=raw[0:64, :]
                                    )
                                    # ro = raw*cos + sw*[-sin;+sin]
                                    ct = tpool.tile([128, 512], BF16, tag="qct")
                                    nc.vector.tensor_mul(
                                        sw[:], sw[:], sin_sb[:, nsl]
                                    )
                                    nc.vector.tensor_mul(
                                        ct[:], raw[:], cos_sb[:, nsl]
                                    )
                                    nc.vector.tensor_add(
                                        qk_ro[m][:, nsl], ct[:], sw[:]
                                    )

                            # v: [T, GC] untransposed, bias via K=1 matmul
                            for nd in range(4):
                                ndsl = slice(nd * 256, (nd + 1) * 256)
                                wvc = wvpool.tile([128, KT * 256], BF16, tag="wvc")
                                nc.sync.dma_start(
                                    out=wvc[:].rearrange("p (k c) -> p k c", k=KT),
                                    in_=wv[:, ndsl].rearrange(
                                        "(k p) c -> p k c", p=128
                                    ),
                                )
                                for t in range(TT):
                                    ps = pspool.tile([128, 256], F32, tag="pv")
                                    for k in range(KT):
                                        nc.tensor.matmul(
                                            ps[:],
                                            xt_sb[k][:, t * 128:(t + 1) * 128],
                                            wvc[:, k * 256:(k + 1) * 256],
                                            start=(k == 0),
                                            stop=False,
                                        )
                                    nc.tensor.matmul(
                                        ps[:],
                                        ones128_sb[:],
                                        bv_sb[:, ndsl],
                                        start=False,
                                        stop=True,
                                    )
                                    vt = tpool.tile([128, 256], BF16, tag="vt")
                                    nc.vector.tensor_copy(vt[:], ps[:])
                                    nc.sync.dma_start(out=v_d[t][:, ndsl], in_=vt[:])

                        # ---- Phase 2: attention per head ----
                        with tc.tile_pool(name="p2v", bufs=2) as vpool, \
                             tc.tile_pool(name="p2e", bufs=4) as epool, \
                             tc.tile_pool(name="p2n", bufs=2) as npool, \
                             tc.tile_pool(name="p2ps", bufs=2, space="PSUM") as ps2, \
                             tc.tile_pool(name="p2po", bufs=2, space="PSUM") as po2:
                            for h in range(HG):
                                qr = qk_ro[h]
                                kr = qk_ro[8 + h]
                                vh = vpool.tile([128, TT * 128], BF16, tag="vh")
                                for t in range(TT):
                                    nc.sync.dma_start(
                                        out=vh[:, t * 128:(t + 1) * 128],
                                        in_=v_d[t][:, h * 128:(h + 1) * 128],
                                    )
                                for n in range(TC):
                                    ps_o = po2.tile([128, 512], F32, tag="po")
                                    ps_z = po2.tile([1, 512], F32, tag="pz")
                                    qsl = slice(n * 512, (n + 1) * 512)
                                    # full (non-diagonal) k-tiles in pairs:
                                    # two score chunks into one 2-bank psum
                                    # tile, ONE 1024-wide exp
                                    pexps = []   # (pexp_ap, col_off, width, j)
                                    for jp in range(2 * n):
                                        ps_s = ps2.tile([128, 1024], F32, tag="ps")
                                        for u in range(2):
                                            j = 2 * jp + u
                                            half = slice(u * 512, (u + 1) * 512)
                                            nc.tensor.matmul(
                                                ps_s[:, half],
                                                kr[:, j * 128:(j + 1) * 128],
                                                qr[:, qsl],
                                                start=True,
                                                stop=True,
                                            )
                                        pexp = epool.tile(
                                            [128, 1024], BF16, tag="pexp"
                                        )
                                        nc.scalar.activation(
                                            pexp[:],
                                            ps_s[:],
                                            mybir.ActivationFunctionType.Exp,
                                            scale=SCALE,
                                        )
                                        for u in range(2):
                                            j = 2 * jp + u
                                            pexps.append(
                                                (pexp, u * 512, 512, 0, j)
                                            )
                                    # diagonal group: narrowed to the causal
                                    # trapezoid
                                    ps_d = [
                                        ps2.tile([128, 1024], F32, tag="ps",
                                                 name="psd0"),
                                        ps2.tile([128, 1024], F32, tag="ps",
                                                 name="psd1"),
                                    ]
                                    for r in range(4):
                                        j = 4 * n + r
                                        w_r = DIAG_W[r]
                                        dsl = slice(DIAG_OFF[r], DIAG_OFF[r] + w_r)
                                        nc.tensor.matmul(
                                            ps_d[DIAG_TILE[r]][:, dsl],
                                            kr[:, j * 128:(j + 1) * 128],
                                            qr[:, n * 512 + 128 * r:(n + 1) * 512],
                                            start=True,
                                            stop=False,
                                        )
                                        nc.tensor.matmul(
                                            ps_d[DIAG_TILE[r]][:, dsl],
                                            ident_sb[:],
                                            masks_sb[:, r * 512 + 128 * r:
                                                     (r + 1) * 512],
                                            start=False,
                                            stop=True,
                                        )
                                    pexp_d = [
                                        epool.tile([128, 1024], BF16, tag="pexp",
                                                   name="pexpd0"),
                                        epool.tile([128, 1024], BF16, tag="pexp",
                                                   name="pexpd1"),
                                    ]
                                    nc.scalar.activation(
                                        pexp_d[0][:, 0:896],
                                        ps_d[0][:, 0:896],
                                        mybir.ActivationFunctionType.Exp,
                                        scale=SCALE,
                                    )
                                    nc.scalar.activation(
                                        pexp_d[1][:, 0:384],
                                        ps_d[1][:, 0:384],
                                        mybir.ActivationFunctionType.Exp,
                                        scale=SCALE,
                                    )
                                    for r in range(4):
                                        pexps.append(
                                            (pexp_d[DIAG_TILE[r]], DIAG_OFF[r],
                                             DIAG_W[r], 128 * r, 4 * n + r)
                                        )
                                    # P@V and Z accumulation
                                    njj = len(pexps)
                                    for idx, (pe_t, off, w_r, qo, j) in enumerate(
                                        pexps
                                    ):
                                        psl = slice(off, off + w_r)
                                        osl = slice(qo, 512)
                                        nc.tensor.matmul(
                                            ps_o[:, osl],
                                            vh[:, j * 128:(j + 1) * 128],
                                            pe_t[:, psl],
                                            start=(idx == 0),
                                            stop=(idx == njj - 1),
                                        )
                                        nc.tensor.matmul(
                                            ps_z[:, osl],
                                            onescol_sb[:],
                                            pe_t[:, psl],
                                            start=(idx == 0),
                                            stop=(idx == njj - 1),
                                        )
                                    rz = npool.tile([1, 512], F32, tag="rz")
                                    nc.vector.reciprocal(rz[:], ps_z[:])
                                    rzb = npool.tile([128, 512], F32, tag="rzb")
                                    nc.gpsimd.partition_broadcast(rzb[:], rz[:])
                                    nc.vector.tensor_mul(
                                        attn_sb[h][:, qsl],
                                        ps_o[:],
                                        rzb[:],
                                    )

                    # ---- Phase 3: out projection ----
                    with tc.tile_pool(name="p3w", bufs=2) as wpool3, \
                         tc.tile_pool(name="p3t", bufs=3) as tpool3, \
                         tc.tile_pool(name="p3ps", bufs=2, space="PSUM") as ps3:
                        for n in range(4):
                            woc = wpool3.tile([128, HG * 512], BF16, tag="woc")
                            nc.sync.dma_start(
                                out=woc[:].rearrange("p (h c) -> p h c", h=HG),
                                in_=wo[:, n * 512:(n + 1) * 512].rearrange(
                                    "(h p) c -> p h c", p=128
                                ),
                            )
                            for m in range(TT):
                                ps_y = ps3.tile([128, 512], F32, tag="py")
                                for h in range(HG):
                                    nc.tensor.matmul(
                                        ps_y[:],
                                        attn_sb[h][:, m * 128:(m + 1) * 128],
                                        woc[:, h * 512:(h + 1) * 512],
                                        start=(h == 0),
                                        stop=(h == HG - 1),
                                    )
                                yt = tpool3.tile([128, 512], BF16, tag="yt")
                                nc.scalar.copy(yt[:], ps_y[:])
                                nc.sync.dma_start(
                                    out=y[m * 128:(m + 1) * 128,
                                          n * 512:(n + 1) * 512],
                                    in_=yt[:],
                                )

            if iters == 1:
                full_body(None)
            else:
                with tc.For_i(0, iters, 1) as iv:
                    full_body(iv)

    nc.compile()
    return nc


def make_host_inputs(x, Wqkv, bqkv, Wo):
    """Per-core input maps (host-side sharding)."""
    half = D // 2
    freq = np.arange(half, dtype=np.float64)
    theta = 1.0 / (10000.0 ** (2.0 * freq / D))
    pos = np.arange(T, dtype=np.float64)
    ang = pos[:, None] * theta[None, :]          # [T, half]
    sinT = np.sin(ang).T.astype(np.float32)      # [half, T]
    cosT = np.cos(ang).T.astype(np.float32)
    # sign folded into the sin table for the partition-swap RoPE form
    sin2 = np.concatenate([-sinT, sinT], axis=0).astype(NPBF16)  # [128, T]
    cos2 = np.concatenate([cosT, cosT], axis=0).astype(NPBF16)

    masks = np.zeros((4, 128, 512), dtype=np.float32)
    f = np.arange(512)[None, :]
    p = np.arange(128)[:, None]
    for r in range(4):
        masks[r] = np.where(f >= r * 128 + p, 0.0, MASKVAL)
    masks = masks.astype(NPBF16)
    ident = np.eye(128, dtype=np.float32).astype(NPBF16)
    onescol = np.ones((128, 1), dtype=NPBF16)
    ones128 = np.ones((1, 128), dtype=NPBF16)

    xT = [np.ascontiguousarray(x[b].T).astype(NPBF16) for b in range(B)]
    in_maps = []
    for core in range(N_CORES):
        b, g = core // 2, core % 2
        cs = slice(g * GC, (g + 1) * GC)
        in_maps.append({
            "xT": xT[b],
            "wq": np.ascontiguousarray(Wqkv[:, :C][:, cs]).astype(NPBF16),
            "wk": np.ascontiguousarray(Wqkv[:, C:2 * C][:, cs]).astype(NPBF16),
            "wv": np.ascontiguousarray(Wqkv[:, 2 * C:][:, cs]).astype(NPBF16),
            "bq": np.ascontiguousarray(bqkv[:C][cs].reshape(GC, 1)),
            "bk": np.ascontiguousarray(bqkv[C:2 * C][cs].reshape(GC, 1)),
            "bv": np.ascontiguousarray(bqkv[2 * C:][cs].reshape(1, GC)).astype(NPBF16),
            "wo": np.ascontiguousarray(Wo[cs, :]).astype(NPBF16),
            "sin2": sin2,
            "cos2": cos2,
            "masks": masks,
            "ident": ident,
            "onescol": onescol,
            "ones128": ones128,
        })
    return in_maps


_PROGRAM_CACHE = {}


def get_program(iters=1):
    if iters not in _PROGRAM_CACHE:
        _PROGRAM_CACHE[iters] = build_program(iters)
    return _PROGRAM_CACHE[iters]


def kernel(x, Wqkv, bqkv, Wo, bo):
    x = np.asarray(x, dtype=np.float32)
    Wqkv = np.asarray(Wqkv, dtype=np.float32)
    bqkv = np.asarray(bqkv, dtype=np.float32)
    Wo = np.asarray(Wo, dtype=np.float32)
    bo = np.asarray(bo, dtype=np.float32)

    nc = get_program(1)
    in_maps = make_host_inputs(x, Wqkv, bqkv, Wo)
    res = run_bass_kernel_spmd(nc, in_maps, list(range(N_CORES)))

    out = np.empty((B, T, C), dtype=np.float32)
    for b in range(B):
        out[b] = (res.results[2 * b]["y"].astype(np.float32)
                  + res.results[2 * b + 1]["y"].astype(np.float32) + bo)
    return out
